# revision 33
# baseline (speedup 1.0000x reference)
"""MLAttention (label-pooling attention) Trainium2 Bass kernel.

Computes, for full inputs:
    scores = einsum('bsh,lh->bls', inputs, W)
    scores = where(mask==0, -inf, scores)
    attn   = softmax(scores, axis=-1)
    out    = einsum('bls,bsh->blh', attn, inputs)

Label-parallel across 8 NeuronCores: L=28415 padded to 28672 = 8*3584.
Each core gets its own W shard [3584, 512]; inputs/masks replicated.
Host concatenates the 8 per-core outputs [B, 3584, H] and trims to L.

Matmul operands live in SBUF as float32r (same 32-bit data; the producing
DVE copy rounds) so the PE runs single-pass full-rate matmuls instead of
fp32's two half-speed LOW/HIGH passes.

Per-core dataflow:
  setup:  host passes x, x^T, and the W-shard transpose; the kernel DMAs
          them into fp32 staging and DVE-rounds into resident f32r SBUF
          tensors (XB [S,H]-chunked, XT [H,S]-chunked, WT [H,L]); W
          chunks stream in two chunks ahead of the b=0 compute pass.
          Mask rows are replicated across partitions via a K=1 matmul.
  main:   per (b, 128-label tile):
            PE   : scores_psum = sum_k WT_k^T @ XT_k      (4 matmuls N=512)
            ACT  : exp_sbuf    = Exp(scores_psum)
            DVE  : expm = exp*maskrep + rowsum (one scalar_tensor_tensor)
            DVE  : recip = 1/rowsum
            PE   : expT_psum   = transpose(expm)           (4 transposes)
            DVE  : expT_sbuf   = copy(expT_psum)
            PE   : out_psum    = sum_s expT_s^T @ X_s      (4 matmuls N=512)
            ACT  : out_sbuf    = Copy(out_psum * recip)    (per-label scale)
            DMA  : out_sbuf -> out[b, tile, :]
"""

from contextlib import ExitStack

import numpy as np

import concourse.bass as bass
import concourse.mybir as mybir
import concourse.tile as tile
from concourse import bacc, bass_utils
from concourse.bass import ds, ts
from concourse.masks import make_identity

F32 = mybir.dt.float32

# Problem shapes (hardcoded per contract).
B, S, H, L = 4, 512, 512, 28415
N_CORES = 8
LSH = 3584               # per-core padded label count (28 tiles of 128)
L_PAD = LSH * N_CORES    # 28672


def build_module(b_sz=B, s_sz=S, h_sz=H, lsh=LSH, n_devices=N_CORES, mm_dt=None):
    """Build the per-core Bass/Tile module (SPMD: same program, per-core data)."""
    if mm_dt is None:
        mm_dt = mybir.dt.float32r  # full-rate PE mode for N>=256 fp32 matmuls
    P = 128
    KH = h_sz // P   # H contraction chunks
    KS = s_sz // P   # S contraction chunks
    NT = lsh // P    # label tiles per core

    nc = bacc.Bacc(
        "TRN2", target_bir_lowering=False, debug=False, num_devices=n_devices
    )
    x_d = nc.dram_tensor("x", [b_sz, s_sz, h_sz], F32, kind="ExternalInput").ap()
    xt_d = nc.dram_tensor("xt", [b_sz, h_sz, s_sz], F32, kind="ExternalInput").ap()
    wt_d = nc.dram_tensor("wt", [h_sz, lsh], F32, kind="ExternalInput").ap()
    m_d = nc.dram_tensor("m", [b_sz, s_sz], F32, kind="ExternalInput").ap()
    o_d = nc.dram_tensor("o", [b_sz, lsh, h_sz], F32, kind="ExternalOutput").ap()

    with tile.TileContext(nc) as tc, ExitStack() as ctx:
        const = ctx.enter_context(tc.tile_pool(name="const", bufs=1))
        res = ctx.enter_context(tc.tile_pool(name="res", bufs=1))
        work = ctx.enter_context(tc.tile_pool(name="work", bufs=3))
        psum = ctx.enter_context(tc.tile_pool(name="psum", bufs=2, space="PSUM"))

        ident = const.tile([P, P], F32)
        make_identity(nc, ident[:])
        ident_r = const.tile([P, P], mm_dt)
        nc.vector.tensor_copy(ident_r[:], ident[:])
        zbias = const.tile([P, 1], F32)
        nc.gpsimd.memset(zbias[:], 0.0)
        ones_row = const.tile([1, P], F32)
        nc.gpsimd.memset(ones_row[:], 1.0)

        # Resident SBUF tensors. Matmul operands are float32r (same bits as
        # fp32; the producing copy rounds) so the PE runs single-pass
        # full-rate matmuls instead of fp32's 2x half-speed passes.
        WT = res.tile([P, KH, lsh], mm_dt)        # WT[h%128, h//128, l] = W[l, h]
        XB = res.tile([P, b_sz, KS, h_sz], mm_dt)  # XB[s%128, b, s//128, h]
        XT = res.tile([P, b_sz, KH, s_sz], mm_dt)  # XT[h%128, b, h//128, s]
        MR = res.tile([P, b_sz, s_sz], F32)      # mask row replicated over partitions

        def x_setup(b):
            """Stage inputs[b] (natural + host-transposed), round into XB/XT."""
            xstage = work.tile([P, KS, h_sz], F32, tag="xstage", bufs=2)
            nc.sync.dma_start(
                xstage[:], x_d[b].rearrange("(c p) h -> p c h", p=P)
            )
            nc.vector.tensor_copy(XB[:, b], xstage[:])
            xtstage = work.tile([P, KH, s_sz], F32, tag="xtstage", bufs=2)
            nc.sync.dma_start(
                xtstage[:], xt_d[b].rearrange("(k p) s -> p k s", p=P)
            )
            nc.vector.tensor_copy(XT[:, b], xtstage[:])

        def mask_setup(b):
            """Replicate mask row across partitions via K=1 matmul with ones."""
            mrow = work.tile([1, s_sz], F32, tag="mrow")
            nc.sync.dma_start(mrow[:], m_d[b : b + 1, :])
            pm = psum.tile([P, s_sz], F32, tag="ps_sc", bufs=2)
            nc.tensor.matmul(pm[:], ones_row[:], mrow[:], start=True, stop=True)
            nc.vector.tensor_copy(MR[:, b, :], pm[:])

        WCH = min(512, lsh)  # W label-chunk per DMA+cast
        NW = lsh // WCH

        def w_setup(g):
            """Load one host-transposed W chunk [H, 512] and round into WT."""
            wstage = work.tile([P, KH, WCH], F32, tag="wstage", bufs=2)
            nc.sync.dma_start(
                wstage[:],
                wt_d[:, ts(g, WCH)].rearrange("(k p) l -> p k l", p=P),
            )
            nc.vector.tensor_copy(WT[:, :, ts(g, WCH)], wstage[:])

        def main_tile(b, t):
            ps_sc = psum.tile([P, s_sz], F32, tag="ps_sc", bufs=2)
            for k in range(KH):
                nc.tensor.matmul(
                    ps_sc[:],
                    WT[:, k, ts(t, P)],
                    XT[:, b, k, :],
                    start=(k == 0),
                    stop=(k == KH - 1),
                )

            exp_t = work.tile([P, s_sz], F32, tag="exp")
            nc.scalar.activation(
                exp_t[:], ps_sc[:], mybir.ActivationFunctionType.Exp,
                bias=zbias[:],
            )

            # Mask + row-sum in one DVE pass.
            expm = work.tile([P, s_sz], mm_dt, tag="expm")
            rowsum = work.tile([P, 1], F32, tag="rowsum")
            nc.vector.scalar_tensor_tensor(
                out=expm[:],
                in0=exp_t[:],
                scalar=1.0,
                in1=MR[:, b, :],
                op0=mybir.AluOpType.mult,
                op1=mybir.AluOpType.mult,
                accum_out=rowsum[:],
            )
            recip = work.tile([P, 1], F32, tag="recip")
            nc.vector.reciprocal(recip[:], rowsum[:])

            ps_tr = psum.tile([P, KS, P], mm_dt, tag="ps_tr")
            for c in range(KS):
                nc.tensor.transpose(
                    ps_tr[:, c, :], expm[:, ts(c, P)], ident_r[:]
                )
            expT = work.tile([P, KS, P], mm_dt, tag="expT")
            nc.vector.tensor_copy(expT[:], ps_tr[:])

            ps_out = psum.tile([P, h_sz], F32, tag="ps_out")
            for c in range(KS):
                nc.tensor.matmul(
                    ps_out[:],
                    expT[:, c, :],
                    XB[:, b, c, :],
                    start=(c == 0),
                    stop=(c == KS - 1),
                )

            out_t = work.tile([P, h_sz], F32, tag="out")
            nc.scalar.activation(
                out_t[:], ps_out[:], mybir.ActivationFunctionType.Copy,
                scale=recip[:],
            )
            nc.sync.dma_start(o_d[b, ts(t, P), :], out_t[:])

        # ---- emission order tuned for DMA pipelining + PE warm-up:
        # mask + b=0 inputs first, then the b=0 label pass with W chunk
        # loads staying two chunks ahead of consumption, then the
        # remaining batches (inputs DMA'd during the b=0 pass).
        for b in range(b_sz):
            mask_setup(b)
        x_setup(0)
        for g in range(min(2, NW)):
            w_setup(g)
        for t in range(NT):
            if t % (WCH // P) == 0:
                g = t // (WCH // P) + 2
                if g < NW:
                    w_setup(g)
            main_tile(0, t)
        for b in range(1, b_sz):
            x_setup(b)
        for b in range(1, b_sz):
            for t in range(NT):
                main_tile(b, t)

    nc.compile()
    return nc


def build_module_b(b_sz=B, s_sz=S, h_sz=H, lsh=LSH, n_devices=N_CORES, mm_dt=None):
    """Transposed-scores variant: scores computed in [S, L] layout so the
    exp tile is directly the mm2 stationary (no per-tile PE transposes).
    Row-sums via a ones-vector matmul; mask folded into the exp bias."""
    if mm_dt is None:
        mm_dt = mybir.dt.float32r
    P = 128
    KH = h_sz // P
    KS = s_sz // P
    LG = min(512, lsh)     # label group per mm1 sweep
    NG = lsh // LG
    NSUB = LG // P         # 128-label subtiles per group

    nc = bacc.Bacc(
        "TRN2", target_bir_lowering=False, debug=False, num_devices=n_devices
    )
    x_d = nc.dram_tensor("x", [b_sz, s_sz, h_sz], F32, kind="ExternalInput").ap()
    xt_d = nc.dram_tensor("xt", [b_sz, h_sz, s_sz], F32, kind="ExternalInput").ap()
    wt_d = nc.dram_tensor("wt", [h_sz, lsh], F32, kind="ExternalInput").ap()
    m_d = nc.dram_tensor("m", [b_sz, s_sz], F32, kind="ExternalInput").ap()
    o_d = nc.dram_tensor("o", [b_sz, lsh, h_sz], F32, kind="ExternalOutput").ap()

    with tile.TileContext(nc) as tc, ExitStack() as ctx:
        const = ctx.enter_context(tc.tile_pool(name="const", bufs=1))
        res = ctx.enter_context(tc.tile_pool(name="res", bufs=1))
        work = ctx.enter_context(tc.tile_pool(name="work", bufs=3))
        psum = ctx.enter_context(tc.tile_pool(name="psum", bufs=2, space="PSUM"))

        ones_f = const.tile([P, 2], F32)
        nc.gpsimd.memset(ones_f[:], 1.0)
        ones2 = const.tile([P, 2], mm_dt)
        nc.vector.tensor_copy(ones2[:], ones_f[:])
        one_11 = const.tile([1, 1], F32)
        nc.gpsimd.memset(one_11[:], 1.0)

        WT = res.tile([P, KH, lsh], mm_dt)
        XB = res.tile([P, b_sz, KS, h_sz], mm_dt)
        XT = res.tile([P, b_sz, KH, s_sz], mm_dt)
        MB = res.tile([P, b_sz, KS], F32)   # exp bias: (mask-1)*30 per s

        def mask_setup():
            mbr = work.tile([P, b_sz, KS], F32, tag="mbr")
            nc.sync.dma_start(mbr[:], m_d.rearrange("b (c p) -> p b c", p=P))
            nc.vector.tensor_scalar_mul(out=mbr[:], in0=mbr[:], scalar1=30.0)
            nc.vector.tensor_scalar_add(out=MB[:], in0=mbr[:], scalar1=-30.0)

        def x_setup(b):
            xstage = work.tile([P, KS, h_sz], F32, tag="stage", bufs=3)
            nc.sync.dma_start(
                xstage[:], x_d[b].rearrange("(c p) h -> p c h", p=P)
            )
            nc.vector.tensor_copy(XB[:, b], xstage[:])
            xtstage = work.tile([P, KH, s_sz], F32, tag="stage", bufs=3)
            nc.sync.dma_start(
                xtstage[:], xt_d[b].rearrange("(k p) s -> p k s", p=P)
            )
            nc.vector.tensor_copy(XT[:, b], xtstage[:])

        def w_setup(g):
            wstage = work.tile([P, KH, LG], F32, tag="stage", bufs=3)
            nc.sync.dma_start(
                wstage[:],
                wt_d[:, ts(g, LG)].rearrange("(k p) l -> p k l", p=P),
            )
            nc.vector.tensor_copy(WT[:, :, ts(g, LG)], wstage[:])

        def main_group(b, g):
            # mm1: scoresT chunks [128 S, LG] accumulated over H
            ps_sct = psum.tile([P, KS, LG], F32, tag="ps_sct", bufs=1)
            for sc in range(KS):
                for k in range(KH):
                    nc.tensor.matmul(
                        ps_sct[:, sc, :],
                        XT[:, b, k, ts(sc, P)],
                        WT[:, k, ts(g, LG)],
                        start=(k == 0),
                        stop=(k == KH - 1),
                    )
            # exp with mask bias (per-partition = per-s), rounded to f32r
            exp_g = work.tile([P, KS, LG], mm_dt, tag="exp_g", bufs=2)
            for sc in range(KS):
                nc.scalar.activation(
                    exp_g[:, sc, :], ps_sct[:, sc, :],
                    mybir.ActivationFunctionType.Exp,
                    bias=MB[:, b, sc : sc + 1],
                )
            # label sums via ones-vector matmul (partition reduce on PE)
            ps_sum = psum.tile([2, LG], F32, tag="ps_sum", bufs=1)
            for sc in range(KS):
                nc.tensor.matmul(
                    ps_sum[:], ones2[:], exp_g[:, sc, :],
                    start=(sc == 0), stop=(sc == KS - 1),
                )
            sums_row = work.tile([1, LG], F32, tag="sums_row")
            nc.vector.tensor_copy(sums_row[:], ps_sum[0:1, :])

            ps_out = [None] * NSUB
            out_t = [None] * NSUB

            def mm2(l):
                ps_out[l] = psum.tile([P, h_sz], F32, tag="ps_out", bufs=2, name="ps_out_b")
                for sc in range(KS):
                    nc.tensor.matmul(
                        ps_out[l][:],
                        exp_g[:, sc, ts(l, P)],
                        XB[:, b, sc, :],
                        start=(sc == 0),
                        stop=(sc == KS - 1),
                    )

            mm2(0)
            # transpose sums row -> per-label columns via K=1 matmuls
            ps_tiny = psum.tile([P, NSUB], F32, tag="ps_tiny", bufs=1)
            for l in range(NSUB):
                nc.tensor.matmul(
                    ps_tiny[:, l : l + 1], sums_row[:, ts(l, P)], one_11[:],
                    start=True, stop=True,
                )
            sums_col = work.tile([P, NSUB], F32, tag="sums_col")
            nc.vector.tensor_copy(sums_col[:], ps_tiny[:])
            recips = work.tile([P, NSUB], F32, tag="recips")
            nc.vector.reciprocal(recips[:], sums_col[:])

            def finish(l):
                out_t[l] = work.tile([P, h_sz], F32, tag="out", name="out_b")
                nc.scalar.activation(
                    out_t[l][:], ps_out[l][:],
                    mybir.ActivationFunctionType.Copy,
                    scale=recips[:, l : l + 1],
                )
                nc.sync.dma_start(
                    o_d[b, ds(g * LG + l * P, P), :], out_t[l][:]
                )

            finish(0)
            for l in range(1, NSUB):
                mm2(l)
                finish(l)

        mask_setup()
        x_setup(0)
        for g in range(min(2, NG)):
            w_setup(g)
        for g in range(NG):
            if g + 2 < NG:
                w_setup(g + 2)
            main_group(0, g)
        for b in range(1, b_sz):
            x_setup(b)
        for b in range(1, b_sz):
            for g in range(NG):
                main_group(b, g)

    nc.compile()
    return nc


_CACHE = {}

VARIANT = "a"  # "a": per-tile PE transposes (v7); "b": transposed-scores


def _get_module():
    if VARIANT not in _CACHE:
        _CACHE[VARIANT] = (
            build_module() if VARIANT == "a" else build_module_b()
        )
    return _CACHE[VARIANT]


def _run(inputs: np.ndarray, masks: np.ndarray, W: np.ndarray, **spmd_kwargs):
    """Run on 8 cores; returns (full output, BassKernelResults)."""
    nc = _get_module()

    x = np.ascontiguousarray(inputs, dtype=np.float32)
    xt = np.ascontiguousarray(np.swapaxes(x, 1, 2))
    mf = np.ascontiguousarray(masks, dtype=np.float32)
    wt_pad = np.zeros((H, L_PAD), dtype=np.float32)
    wt_pad[:, :L] = W.T

    in_maps = [
        {
            "x": x,
            "xt": xt,
            "m": mf,
            "wt": np.ascontiguousarray(wt_pad[:, c * LSH : (c + 1) * LSH]),
        }
        for c in range(N_CORES)
    ]
    res = bass_utils.run_bass_kernel_spmd(
        nc, in_maps, core_ids=list(range(N_CORES)), **spmd_kwargs
    )
    out = np.concatenate([res.results[c]["o"] for c in range(N_CORES)], axis=1)
    return np.ascontiguousarray(out[:, :L, :]), res


def kernel(inputs: np.ndarray, masks: np.ndarray, W: np.ndarray) -> np.ndarray:
    out, _ = _run(inputs, masks, W)
    return out


# revision 37
# speedup vs baseline: 1.1561x; 1.1561x over previous
"""MLAttention (label-pooling attention) Trainium2 Bass kernel.

Computes, for full inputs:
    scores = einsum('bsh,lh->bls', inputs, W)
    scores = where(mask==0, -inf, scores)
    attn   = softmax(scores, axis=-1)
    out    = einsum('bls,bsh->blh', attn, inputs)

Label-parallel across 8 NeuronCores: L=28415 padded to 28672 = 8*3584.
Each core gets its own W shard [3584, 512]; inputs/masks replicated.
Host concatenates the 8 per-core outputs [B, 3584, H] and trims to L.

Matmul operands live in SBUF as float32r (same 32-bit data; the producing
DVE copy rounds) so the PE runs single-pass full-rate matmuls instead of
fp32's two half-speed LOW/HIGH passes.

Per-core dataflow:
  setup:  host passes x, x^T, and the W-shard transpose; the kernel DMAs
          them into fp32 staging and DVE-rounds into resident f32r SBUF
          tensors (XB [S,H]-chunked, XT [H,S]-chunked, WT [H,L]); W
          chunks stream in two chunks ahead of the b=0 compute pass.
          Mask rows are replicated across partitions via a K=1 matmul.
  main:   per (b, 128-label tile):
            PE   : scores_psum = sum_k WT_k^T @ XT_k      (4 matmuls N=512)
            ACT  : exp_sbuf    = Exp(scores_psum)
            DVE  : expm = exp*maskrep + rowsum (one scalar_tensor_tensor)
            DVE  : recip = 1/rowsum
            PE   : expT_psum   = transpose(expm)           (4 transposes)
            DVE  : expT_sbuf   = copy(expT_psum)
            PE   : out_psum    = sum_s expT_s^T @ X_s      (4 matmuls N=512)
            ACT  : out_sbuf    = Copy(out_psum * recip)    (per-label scale)
            DMA  : out_sbuf -> out[b, tile, :]
"""

from contextlib import ExitStack

import numpy as np

import concourse.bass as bass
import concourse.mybir as mybir
import concourse.tile as tile
from concourse import bacc, bass_utils
from concourse.bass import ds, ts
from concourse.masks import make_identity

F32 = mybir.dt.float32

# Problem shapes (hardcoded per contract).
B, S, H, L = 4, 512, 512, 28415
N_CORES = 8
LSH = 3584               # per-core padded label count (28 tiles of 128)
L_PAD = LSH * N_CORES    # 28672


def build_module(b_sz=B, s_sz=S, h_sz=H, lsh=LSH, n_devices=N_CORES, mm_dt=None):
    """Build the per-core Bass/Tile module (SPMD: same program, per-core data)."""
    if mm_dt is None:
        mm_dt = mybir.dt.float32r  # full-rate PE mode for N>=256 fp32 matmuls
    P = 128
    KH = h_sz // P   # H contraction chunks
    KS = s_sz // P   # S contraction chunks
    NT = lsh // P    # label tiles per core

    nc = bacc.Bacc(
        "TRN2", target_bir_lowering=False, debug=False, num_devices=n_devices
    )
    x_d = nc.dram_tensor("x", [b_sz, s_sz, h_sz], F32, kind="ExternalInput").ap()
    xt_d = nc.dram_tensor("xt", [b_sz, h_sz, s_sz], F32, kind="ExternalInput").ap()
    wt_d = nc.dram_tensor("wt", [h_sz, lsh], F32, kind="ExternalInput").ap()
    m_d = nc.dram_tensor("m", [b_sz, s_sz], F32, kind="ExternalInput").ap()
    o_d = nc.dram_tensor("o", [b_sz, lsh, h_sz], F32, kind="ExternalOutput").ap()

    with tile.TileContext(nc) as tc, ExitStack() as ctx:
        const = ctx.enter_context(tc.tile_pool(name="const", bufs=1))
        res = ctx.enter_context(tc.tile_pool(name="res", bufs=1))
        work = ctx.enter_context(tc.tile_pool(name="work", bufs=3))
        psum = ctx.enter_context(tc.tile_pool(name="psum", bufs=2, space="PSUM"))

        ident = const.tile([P, P], F32)
        make_identity(nc, ident[:])
        ident_r = const.tile([P, P], mm_dt)
        nc.vector.tensor_copy(ident_r[:], ident[:])
        zbias = const.tile([P, 1], F32)
        nc.gpsimd.memset(zbias[:], 0.0)
        ones_row = const.tile([1, P], F32)
        nc.gpsimd.memset(ones_row[:], 1.0)

        # Resident SBUF tensors. Matmul operands are float32r (same bits as
        # fp32; the producing copy rounds) so the PE runs single-pass
        # full-rate matmuls instead of fp32's 2x half-speed passes.
        WT = res.tile([P, KH, lsh], mm_dt)        # WT[h%128, h//128, l] = W[l, h]
        XB = res.tile([P, b_sz, KS, h_sz], mm_dt)  # XB[s%128, b, s//128, h]
        XT = res.tile([P, b_sz, KH, s_sz], mm_dt)  # XT[h%128, b, h//128, s]
        MR = res.tile([P, b_sz, s_sz], F32)      # mask row replicated over partitions

        def x_setup(b):
            """Stage inputs[b] (natural + host-transposed), round into XB/XT."""
            xstage = work.tile([P, KS, h_sz], F32, tag="stage", bufs=3)
            nc.sync.dma_start(
                xstage[:], x_d[b].rearrange("(c p) h -> p c h", p=P)
            )
            nc.vector.tensor_copy(XB[:, b], xstage[:])
            xtstage = work.tile([P, KH, s_sz], F32, tag="stage", bufs=3)
            nc.sync.dma_start(
                xtstage[:], xt_d[b].rearrange("(k p) s -> p k s", p=P)
            )
            nc.vector.tensor_copy(XT[:, b], xtstage[:])

        def mask_setup(b):
            """Replicate mask row across partitions via K=1 matmul with ones."""
            mrow = work.tile([1, s_sz], F32, tag="mrow")
            nc.sync.dma_start(mrow[:], m_d[b : b + 1, :])
            pm = psum.tile([P, s_sz], F32, tag="ps_sc", bufs=2)
            nc.tensor.matmul(pm[:], ones_row[:], mrow[:], start=True, stop=True)
            nc.vector.tensor_copy(MR[:, b, :], pm[:])

        WCH = min(512, lsh)  # W label-chunk per DMA+cast
        NW = lsh // WCH

        def w_setup(g):
            """Load one host-transposed W chunk [H, 512] and round into WT.

            Split per k-chunk so the DVE casts spread across the main
            loop's slack instead of stalling the PE in one burst."""
            wstage = work.tile([P, KH, WCH], F32, tag="stage", bufs=3)
            for k in range(KH):
                nc.sync.dma_start(
                    wstage[:, k], wt_d[k * P : (k + 1) * P, ts(g, WCH)]
                )
                nc.vector.tensor_copy(WT[:, k, ts(g, WCH)], wstage[:, k])

        def main_tile(b, t):
            ps_sc = psum.tile([P, s_sz], F32, tag="ps_sc", bufs=2)
            for k in range(KH):
                nc.tensor.matmul(
                    ps_sc[:],
                    WT[:, k, ts(t, P)],
                    XT[:, b, k, :],
                    start=(k == 0),
                    stop=(k == KH - 1),
                )

            exp_t = work.tile([P, s_sz], F32, tag="exp")
            nc.scalar.activation(
                exp_t[:], ps_sc[:], mybir.ActivationFunctionType.Exp,
                bias=zbias[:],
            )

            # Mask + row-sum in one DVE pass.
            expm = work.tile([P, s_sz], mm_dt, tag="expm")
            rowsum = work.tile([P, 1], F32, tag="rowsum")
            nc.vector.scalar_tensor_tensor(
                out=expm[:],
                in0=exp_t[:],
                scalar=1.0,
                in1=MR[:, b, :],
                op0=mybir.AluOpType.mult,
                op1=mybir.AluOpType.mult,
                accum_out=rowsum[:],
            )
            recip = work.tile([P, 1], F32, tag="recip")
            nc.vector.reciprocal(recip[:], rowsum[:])

            ps_tr = psum.tile([P, KS, P], mm_dt, tag="ps_tr")
            for c in range(KS):
                nc.tensor.transpose(
                    ps_tr[:, c, :], expm[:, ts(c, P)], ident_r[:]
                )
            expT = work.tile([P, KS, P], mm_dt, tag="expT")
            nc.vector.tensor_copy(expT[:], ps_tr[:])

            ps_out = psum.tile([P, h_sz], F32, tag="ps_out")
            for c in range(KS):
                nc.tensor.matmul(
                    ps_out[:],
                    expT[:, c, :],
                    XB[:, b, c, :],
                    start=(c == 0),
                    stop=(c == KS - 1),
                )

            out_t = work.tile([P, h_sz], F32, tag="out")
            nc.scalar.activation(
                out_t[:], ps_out[:], mybir.ActivationFunctionType.Copy,
                scale=recip[:],
            )
            nc.sync.dma_start(o_d[b, ts(t, P), :], out_t[:])

        # ---- emission order tuned for DMA pipelining + PE warm-up:
        # mask + b=0 inputs first, then the b=0 label pass with W chunk
        # loads staying two chunks ahead of consumption, then the
        # remaining batches (inputs DMA'd during the b=0 pass).
        for b in range(b_sz):
            mask_setup(b)
        x_setup(0)
        for g in range(min(2, NW)):
            w_setup(g)
        x_mid = min(NT - 1, max(4, (NT * 2) // 3))
        for b in range(b_sz):
            for t in range(NT):
                if b == 0 and t % (WCH // P) == 0:
                    g = t // (WCH // P) + 2
                    if g < NW:
                        w_setup(g)
                if t == x_mid and b + 1 < b_sz:
                    x_setup(b + 1)
                main_tile(b, t)

    nc.compile()
    return nc


def build_module_b(b_sz=B, s_sz=S, h_sz=H, lsh=LSH, n_devices=N_CORES, mm_dt=None):
    """Transposed-scores variant: scores computed in [S, L] layout so the
    exp tile is directly the mm2 stationary (no per-tile PE transposes).
    Row-sums via a ones-vector matmul; mask folded into the exp bias."""
    if mm_dt is None:
        mm_dt = mybir.dt.float32r
    P = 128
    KH = h_sz // P
    KS = s_sz // P
    LG = min(512, lsh)     # label group per mm1 sweep
    NG = lsh // LG
    NSUB = LG // P         # 128-label subtiles per group

    nc = bacc.Bacc(
        "TRN2", target_bir_lowering=False, debug=False, num_devices=n_devices
    )
    x_d = nc.dram_tensor("x", [b_sz, s_sz, h_sz], F32, kind="ExternalInput").ap()
    xt_d = nc.dram_tensor("xt", [b_sz, h_sz, s_sz], F32, kind="ExternalInput").ap()
    wt_d = nc.dram_tensor("wt", [h_sz, lsh], F32, kind="ExternalInput").ap()
    m_d = nc.dram_tensor("m", [b_sz, s_sz], F32, kind="ExternalInput").ap()
    o_d = nc.dram_tensor("o", [b_sz, lsh, h_sz], F32, kind="ExternalOutput").ap()

    with tile.TileContext(nc) as tc, ExitStack() as ctx:
        const = ctx.enter_context(tc.tile_pool(name="const", bufs=1))
        res = ctx.enter_context(tc.tile_pool(name="res", bufs=1))
        work = ctx.enter_context(tc.tile_pool(name="work", bufs=3))
        psum = ctx.enter_context(tc.tile_pool(name="psum", bufs=2, space="PSUM"))

        ones_f = const.tile([P, 2], F32)
        nc.gpsimd.memset(ones_f[:], 1.0)
        ones2 = const.tile([P, 2], mm_dt)
        nc.vector.tensor_copy(ones2[:], ones_f[:])
        one_11 = const.tile([1, 1], F32)
        nc.gpsimd.memset(one_11[:], 1.0)

        WT = res.tile([P, KH, lsh], mm_dt)
        XB = res.tile([P, b_sz, KS, h_sz], mm_dt)
        XT = res.tile([P, b_sz, KH, s_sz], mm_dt)
        MB = res.tile([P, b_sz, KS], F32)   # exp bias: (mask-1)*30 per s

        def mask_setup():
            mbr = work.tile([P, b_sz, KS], F32, tag="mbr")
            nc.sync.dma_start(mbr[:], m_d.rearrange("b (c p) -> p b c", p=P))
            nc.vector.tensor_scalar_mul(out=mbr[:], in0=mbr[:], scalar1=30.0)
            nc.vector.tensor_scalar_add(out=MB[:], in0=mbr[:], scalar1=-30.0)

        def x_setup(b):
            xstage = work.tile([P, KS, h_sz], F32, tag="stage", bufs=3)
            nc.sync.dma_start(
                xstage[:], x_d[b].rearrange("(c p) h -> p c h", p=P)
            )
            nc.vector.tensor_copy(XB[:, b], xstage[:])
            xtstage = work.tile([P, KH, s_sz], F32, tag="stage", bufs=3)
            nc.sync.dma_start(
                xtstage[:], xt_d[b].rearrange("(k p) s -> p k s", p=P)
            )
            nc.vector.tensor_copy(XT[:, b], xtstage[:])

        def w_setup(g):
            wstage = work.tile([P, KH, LG], F32, tag="stage", bufs=3)
            nc.sync.dma_start(
                wstage[:],
                wt_d[:, ts(g, LG)].rearrange("(k p) l -> p k l", p=P),
            )
            nc.vector.tensor_copy(WT[:, :, ts(g, LG)], wstage[:])

        def main_group(b, g):
            # mm1: scoresT chunks [128 S, LG] accumulated over H
            ps_sct = psum.tile([P, KS, LG], F32, tag="ps_sct", bufs=1)
            for sc in range(KS):
                for k in range(KH):
                    nc.tensor.matmul(
                        ps_sct[:, sc, :],
                        XT[:, b, k, ts(sc, P)],
                        WT[:, k, ts(g, LG)],
                        start=(k == 0),
                        stop=(k == KH - 1),
                    )
            # exp with mask bias (per-partition = per-s), rounded to f32r
            exp_g = work.tile([P, KS, LG], mm_dt, tag="exp_g", bufs=2)
            for sc in range(KS):
                nc.scalar.activation(
                    exp_g[:, sc, :], ps_sct[:, sc, :],
                    mybir.ActivationFunctionType.Exp,
                    bias=MB[:, b, sc : sc + 1],
                )
            # label sums via ones-vector matmul (partition reduce on PE)
            ps_sum = psum.tile([2, LG], F32, tag="ps_sum", bufs=1)
            for sc in range(KS):
                nc.tensor.matmul(
                    ps_sum[:], ones2[:], exp_g[:, sc, :],
                    start=(sc == 0), stop=(sc == KS - 1),
                )
            sums_row = work.tile([1, LG], F32, tag="sums_row")
            nc.vector.tensor_copy(sums_row[:], ps_sum[0:1, :])

            ps_out = [None] * NSUB
            out_t = [None] * NSUB

            def mm2(l):
                ps_out[l] = psum.tile([P, h_sz], F32, tag="ps_out", bufs=2, name="ps_out_b")
                for sc in range(KS):
                    nc.tensor.matmul(
                        ps_out[l][:],
                        exp_g[:, sc, ts(l, P)],
                        XB[:, b, sc, :],
                        start=(sc == 0),
                        stop=(sc == KS - 1),
                    )

            mm2(0)
            # transpose sums row -> per-label columns via K=1 matmuls
            ps_tiny = psum.tile([P, NSUB], F32, tag="ps_tiny", bufs=1)
            for l in range(NSUB):
                nc.tensor.matmul(
                    ps_tiny[:, l : l + 1], sums_row[:, ts(l, P)], one_11[:],
                    start=True, stop=True,
                )
            sums_col = work.tile([P, NSUB], F32, tag="sums_col")
            nc.vector.tensor_copy(sums_col[:], ps_tiny[:])
            recips = work.tile([P, NSUB], F32, tag="recips")
            nc.vector.reciprocal(recips[:], sums_col[:])

            def finish(l):
                out_t[l] = work.tile([P, h_sz], F32, tag="out", name="out_b")
                nc.scalar.activation(
                    out_t[l][:], ps_out[l][:],
                    mybir.ActivationFunctionType.Copy,
                    scale=recips[:, l : l + 1],
                )
                nc.sync.dma_start(
                    o_d[b, ds(g * LG + l * P, P), :], out_t[l][:]
                )

            finish(0)
            for l in range(1, NSUB):
                mm2(l)
                finish(l)

        mask_setup()
        x_setup(0)
        for g in range(min(2, NG)):
            w_setup(g)
        for g in range(NG):
            if g + 2 < NG:
                w_setup(g + 2)
            main_group(0, g)
        for b in range(1, b_sz):
            x_setup(b)
        for b in range(1, b_sz):
            for g in range(NG):
                main_group(b, g)

    nc.compile()
    return nc


_CACHE = {}

VARIANT = "a"  # "a": per-tile PE transposes (v7); "b": transposed-scores


def _get_module():
    if VARIANT not in _CACHE:
        _CACHE[VARIANT] = (
            build_module() if VARIANT == "a" else build_module_b()
        )
    return _CACHE[VARIANT]


def _run(inputs: np.ndarray, masks: np.ndarray, W: np.ndarray, **spmd_kwargs):
    """Run on 8 cores; returns (full output, BassKernelResults)."""
    nc = _get_module()

    x = np.ascontiguousarray(inputs, dtype=np.float32)
    xt = np.ascontiguousarray(np.swapaxes(x, 1, 2))
    mf = np.ascontiguousarray(masks, dtype=np.float32)
    wt_pad = np.zeros((H, L_PAD), dtype=np.float32)
    wt_pad[:, :L] = W.T

    in_maps = [
        {
            "x": x,
            "xt": xt,
            "m": mf,
            "wt": np.ascontiguousarray(wt_pad[:, c * LSH : (c + 1) * LSH]),
        }
        for c in range(N_CORES)
    ]
    res = bass_utils.run_bass_kernel_spmd(
        nc, in_maps, core_ids=list(range(N_CORES)), **spmd_kwargs
    )
    out = np.concatenate([res.results[c]["o"] for c in range(N_CORES)], axis=1)
    return np.ascontiguousarray(out[:, :L, :]), res


def kernel(inputs: np.ndarray, masks: np.ndarray, W: np.ndarray) -> np.ndarray:
    out, _ = _run(inputs, masks, W)
    return out


# revision 40
# speedup vs baseline: 1.1678x; 1.0101x over previous
"""MLAttention (label-pooling attention) Trainium2 Bass kernel.

Computes, for full inputs:
    scores = einsum('bsh,lh->bls', inputs, W)
    scores = where(mask==0, -inf, scores)
    attn   = softmax(scores, axis=-1)
    out    = einsum('bls,bsh->blh', attn, inputs)

Label-parallel across 8 NeuronCores: L=28415 padded to 28672 = 8*3584.
Each core gets its own W shard [3584, 512]; inputs/masks replicated.
Host concatenates the 8 per-core outputs [B, 3584, H] and trims to L.

Matmul operands live in SBUF as float32r (same 32-bit data; the producing
DVE copy rounds) so the PE runs single-pass full-rate matmuls instead of
fp32's two half-speed LOW/HIGH passes.

Per-core dataflow:
  setup:  host passes x, x^T, and the W-shard transpose; the kernel DMAs
          them into fp32 staging and DVE-rounds into resident f32r SBUF
          tensors (XB [S,H]-chunked, XT [H,S]-chunked, WT [H,L]); W
          chunks stream in two chunks ahead of the b=0 compute pass.
          Mask rows are replicated across partitions via a K=1 matmul.
  main:   per (b, 128-label tile):
            PE   : scores_psum = sum_k WT_k^T @ XT_k      (4 matmuls N=512)
            ACT  : exp_sbuf    = Exp(scores_psum)
            DVE  : expm = exp*maskrep + rowsum (one scalar_tensor_tensor)
            DVE  : recip = 1/rowsum
            PE   : expT_psum   = transpose(expm)           (4 transposes)
            DVE  : expT_sbuf   = copy(expT_psum)
            PE   : out_psum    = sum_s expT_s^T @ X_s      (4 matmuls N=512)
            ACT  : out_sbuf    = Copy(out_psum * recip)    (per-label scale)
            DMA  : out_sbuf -> out[b, tile, :]
"""

from contextlib import ExitStack

import numpy as np

import concourse.bass as bass
import concourse.mybir as mybir
import concourse.tile as tile
from concourse import bacc, bass_utils
from concourse.bass import ds, ts
from concourse.masks import make_identity

F32 = mybir.dt.float32

# Problem shapes (hardcoded per contract).
B, S, H, L = 4, 512, 512, 28415
N_CORES = 8
LSH = 3584               # per-core padded label count (28 tiles of 128)
L_PAD = LSH * N_CORES    # 28672


def build_module(b_sz=B, s_sz=S, h_sz=H, lsh=LSH, n_devices=N_CORES, mm_dt=None):
    """Build the per-core Bass/Tile module (SPMD: same program, per-core data)."""
    if mm_dt is None:
        mm_dt = mybir.dt.float32r  # full-rate PE mode for N>=256 fp32 matmuls
    P = 128
    KH = h_sz // P   # H contraction chunks
    KS = s_sz // P   # S contraction chunks
    NT = lsh // P    # label tiles per core

    nc = bacc.Bacc(
        "TRN2", target_bir_lowering=False, debug=False, num_devices=n_devices
    )
    x_d = nc.dram_tensor("x", [b_sz, s_sz, h_sz], F32, kind="ExternalInput").ap()
    xt_d = nc.dram_tensor("xt", [b_sz, h_sz, s_sz], F32, kind="ExternalInput").ap()
    wt_d = nc.dram_tensor("wt", [h_sz, lsh], F32, kind="ExternalInput").ap()
    m_d = nc.dram_tensor("m", [b_sz, s_sz], F32, kind="ExternalInput").ap()
    o_d = nc.dram_tensor("o", [b_sz, lsh, h_sz], F32, kind="ExternalOutput").ap()

    with tile.TileContext(nc) as tc, ExitStack() as ctx:
        const = ctx.enter_context(tc.tile_pool(name="const", bufs=1))
        res = ctx.enter_context(tc.tile_pool(name="res", bufs=1))
        work = ctx.enter_context(tc.tile_pool(name="work", bufs=3))
        psum = ctx.enter_context(tc.tile_pool(name="psum", bufs=2, space="PSUM"))

        ident = const.tile([P, P], F32)
        make_identity(nc, ident[:])
        ident_r = const.tile([P, P], mm_dt)
        nc.vector.tensor_copy(ident_r[:], ident[:])
        zbias = const.tile([P, 1], F32)
        nc.gpsimd.memset(zbias[:], 0.0)
        ones_row = const.tile([1, P], F32)
        nc.gpsimd.memset(ones_row[:], 1.0)

        # Resident SBUF tensors. Matmul operands are float32r (same bits as
        # fp32; the producing copy rounds) so the PE runs single-pass
        # full-rate matmuls instead of fp32's 2x half-speed passes.
        WT = res.tile([P, KH, lsh], mm_dt)        # WT[h%128, h//128, l] = W[l, h]
        XB = res.tile([P, b_sz, KS, h_sz], mm_dt)  # XB[s%128, b, s//128, h]
        XT = res.tile([P, b_sz, KH, s_sz], mm_dt)  # XT[h%128, b, h//128, s]
        MR = res.tile([P, b_sz, s_sz], F32)      # mask row replicated over partitions

        def x_setup(b):
            """Stage inputs[b] (natural + host-transposed), round into XB/XT."""
            xstage = work.tile([P, KS, h_sz], F32, tag="xstage", bufs=2)
            nc.sync.dma_start(
                xstage[:], x_d[b].rearrange("(c p) h -> p c h", p=P)
            )
            nc.vector.tensor_copy(XB[:, b], xstage[:])
            xtstage = work.tile([P, KH, s_sz], F32, tag="xtstage", bufs=2)
            nc.sync.dma_start(
                xtstage[:], xt_d[b].rearrange("(k p) s -> p k s", p=P)
            )
            nc.vector.tensor_copy(XT[:, b], xtstage[:])

        def mask_setup(b):
            """Replicate mask row across partitions via K=1 matmul with ones."""
            mrow = work.tile([1, s_sz], F32, tag="mrow")
            nc.sync.dma_start(mrow[:], m_d[b : b + 1, :])
            pm = psum.tile([P, s_sz], F32, tag="ps_sc", bufs=2)
            nc.tensor.matmul(pm[:], ones_row[:], mrow[:], start=True, stop=True)
            nc.vector.tensor_copy(MR[:, b, :], pm[:])

        WCH = min(512, lsh)  # W label-chunk per DMA+cast
        NW = lsh // WCH

        def w_setup(g):
            """Load one host-transposed W chunk [H, 512] and round into WT."""
            wstage = work.tile([P, KH, WCH], F32, tag="wstage", bufs=2)
            nc.sync.dma_start(
                wstage[:],
                wt_d[:, ts(g, WCH)].rearrange("(k p) l -> p k l", p=P),
            )
            nc.vector.tensor_copy(WT[:, :, ts(g, WCH)], wstage[:])

        def tile_front(b, t):
            """mm1 + exp + mask/rowsum + recip for tile (b, t)."""
            ps_sc = psum.tile([P, s_sz], F32, tag="ps_sc", bufs=2)
            for k in range(KH):
                nc.tensor.matmul(
                    ps_sc[:],
                    WT[:, k, ts(t, P)],
                    XT[:, b, k, :],
                    start=(k == 0),
                    stop=(k == KH - 1),
                )

            exp_t = work.tile([P, s_sz], F32, tag="exp", bufs=2)
            nc.scalar.activation(
                exp_t[:], ps_sc[:], mybir.ActivationFunctionType.Exp,
                bias=zbias[:],
            )

            # Mask + row-sum in one DVE pass.
            expm = work.tile([P, s_sz], mm_dt, tag="expm", bufs=3)
            rowsum = work.tile([P, 1], F32, tag="rowsum", bufs=4)
            nc.vector.scalar_tensor_tensor(
                out=expm[:],
                in0=exp_t[:],
                scalar=1.0,
                in1=MR[:, b, :],
                op0=mybir.AluOpType.mult,
                op1=mybir.AluOpType.mult,
                accum_out=rowsum[:],
            )
            recip = work.tile([P, 1], F32, tag="recip", bufs=4)
            nc.vector.reciprocal(recip[:], rowsum[:])
            return expm, recip

        def tile_back(b, t, expm, recip):
            """transposes + mm2 + scaled store for tile (b, t)."""
            ps_tr = psum.tile([P, KS, P], mm_dt, tag="ps_tr")
            for c in range(KS):
                nc.tensor.transpose(
                    ps_tr[:, c, :], expm[:, ts(c, P)], ident_r[:]
                )
            expT = work.tile([P, KS, P], mm_dt, tag="expT")
            nc.vector.tensor_copy(expT[:], ps_tr[:])

            ps_out = psum.tile([P, h_sz], F32, tag="ps_out")
            for c in range(KS):
                nc.tensor.matmul(
                    ps_out[:],
                    expT[:, c, :],
                    XB[:, b, c, :],
                    start=(c == 0),
                    stop=(c == KS - 1),
                )

            out_t = work.tile([P, h_sz], F32, tag="out")
            nc.scalar.activation(
                out_t[:], ps_out[:], mybir.ActivationFunctionType.Copy,
                scale=recip[:],
            )
            nc.sync.dma_start(o_d[b, ts(t, P), :], out_t[:])

        # One-tile software pipeline: tile t+1's mm1 is emitted before
        # tile t's transposes/mm2, so the in-order PE queue always holds
        # independent work while t's exp->mask chain completes on ACT/DVE.
        pend = [None]

        def main_tile(b, t):
            st = tile_front(b, t)
            if pend[0] is not None:
                tile_back(*pend[0])
            pend[0] = (b, t, *st)

        # ---- emission order tuned for DMA pipelining + PE warm-up:
        # mask + b=0 inputs first, then the b=0 label pass with W chunk
        # loads staying two chunks ahead of consumption, then the
        # remaining batches (inputs DMA'd during the b=0 pass).
        for b in range(b_sz):
            mask_setup(b)
        x_setup(0)
        for g in range(min(2, NW)):
            w_setup(g)
        for t in range(NT):
            if t % (WCH // P) == 0:
                g = t // (WCH // P) + 2
                if g < NW:
                    w_setup(g)
            main_tile(0, t)
        for b in range(1, b_sz):
            x_setup(b)
        for b in range(1, b_sz):
            for t in range(NT):
                main_tile(b, t)
        if pend[0] is not None:
            tile_back(*pend[0])

    nc.compile()
    return nc


def build_module_b(b_sz=B, s_sz=S, h_sz=H, lsh=LSH, n_devices=N_CORES, mm_dt=None):
    """Transposed-scores variant: scores computed in [S, L] layout so the
    exp tile is directly the mm2 stationary (no per-tile PE transposes).
    Row-sums via a ones-vector matmul; mask folded into the exp bias."""
    if mm_dt is None:
        mm_dt = mybir.dt.float32r
    P = 128
    KH = h_sz // P
    KS = s_sz // P
    LG = min(512, lsh)     # label group per mm1 sweep
    NG = lsh // LG
    NSUB = LG // P         # 128-label subtiles per group

    nc = bacc.Bacc(
        "TRN2", target_bir_lowering=False, debug=False, num_devices=n_devices
    )
    x_d = nc.dram_tensor("x", [b_sz, s_sz, h_sz], F32, kind="ExternalInput").ap()
    xt_d = nc.dram_tensor("xt", [b_sz, h_sz, s_sz], F32, kind="ExternalInput").ap()
    wt_d = nc.dram_tensor("wt", [h_sz, lsh], F32, kind="ExternalInput").ap()
    m_d = nc.dram_tensor("m", [b_sz, s_sz], F32, kind="ExternalInput").ap()
    o_d = nc.dram_tensor("o", [b_sz, lsh, h_sz], F32, kind="ExternalOutput").ap()

    with tile.TileContext(nc) as tc, ExitStack() as ctx:
        const = ctx.enter_context(tc.tile_pool(name="const", bufs=1))
        res = ctx.enter_context(tc.tile_pool(name="res", bufs=1))
        work = ctx.enter_context(tc.tile_pool(name="work", bufs=3))
        psum = ctx.enter_context(tc.tile_pool(name="psum", bufs=2, space="PSUM"))

        ones_f = const.tile([P, 2], F32)
        nc.gpsimd.memset(ones_f[:], 1.0)
        ones2 = const.tile([P, 2], mm_dt)
        nc.vector.tensor_copy(ones2[:], ones_f[:])
        one_11 = const.tile([1, 1], F32)
        nc.gpsimd.memset(one_11[:], 1.0)

        WT = res.tile([P, KH, lsh], mm_dt)
        XB = res.tile([P, b_sz, KS, h_sz], mm_dt)
        XT = res.tile([P, b_sz, KH, s_sz], mm_dt)
        MB = res.tile([P, b_sz, KS], F32)   # exp bias: (mask-1)*30 per s

        def mask_setup():
            mbr = work.tile([P, b_sz, KS], F32, tag="mbr")
            nc.sync.dma_start(mbr[:], m_d.rearrange("b (c p) -> p b c", p=P))
            nc.vector.tensor_scalar_mul(out=mbr[:], in0=mbr[:], scalar1=30.0)
            nc.vector.tensor_scalar_add(out=MB[:], in0=mbr[:], scalar1=-30.0)

        def x_setup(b):
            xstage = work.tile([P, KS, h_sz], F32, tag="stage", bufs=3)
            nc.sync.dma_start(
                xstage[:], x_d[b].rearrange("(c p) h -> p c h", p=P)
            )
            nc.vector.tensor_copy(XB[:, b], xstage[:])
            xtstage = work.tile([P, KH, s_sz], F32, tag="stage", bufs=3)
            nc.sync.dma_start(
                xtstage[:], xt_d[b].rearrange("(k p) s -> p k s", p=P)
            )
            nc.vector.tensor_copy(XT[:, b], xtstage[:])

        def w_setup(g):
            wstage = work.tile([P, KH, LG], F32, tag="stage", bufs=3)
            nc.sync.dma_start(
                wstage[:],
                wt_d[:, ts(g, LG)].rearrange("(k p) l -> p k l", p=P),
            )
            nc.vector.tensor_copy(WT[:, :, ts(g, LG)], wstage[:])

        def main_group(b, g):
            # mm1: scoresT chunks [128 S, LG] accumulated over H
            ps_sct = psum.tile([P, KS, LG], F32, tag="ps_sct", bufs=1)
            for sc in range(KS):
                for k in range(KH):
                    nc.tensor.matmul(
                        ps_sct[:, sc, :],
                        XT[:, b, k, ts(sc, P)],
                        WT[:, k, ts(g, LG)],
                        start=(k == 0),
                        stop=(k == KH - 1),
                    )
            # exp with mask bias (per-partition = per-s), rounded to f32r
            exp_g = work.tile([P, KS, LG], mm_dt, tag="exp_g", bufs=2)
            for sc in range(KS):
                nc.scalar.activation(
                    exp_g[:, sc, :], ps_sct[:, sc, :],
                    mybir.ActivationFunctionType.Exp,
                    bias=MB[:, b, sc : sc + 1],
                )
            # label sums via ones-vector matmul (partition reduce on PE)
            ps_sum = psum.tile([2, LG], F32, tag="ps_sum", bufs=1)
            for sc in range(KS):
                nc.tensor.matmul(
                    ps_sum[:], ones2[:], exp_g[:, sc, :],
                    start=(sc == 0), stop=(sc == KS - 1),
                )
            sums_row = work.tile([1, LG], F32, tag="sums_row")
            nc.vector.tensor_copy(sums_row[:], ps_sum[0:1, :])

            ps_out = [None] * NSUB
            out_t = [None] * NSUB

            def mm2(l):
                ps_out[l] = psum.tile([P, h_sz], F32, tag="ps_out", bufs=2, name="ps_out_b")
                for sc in range(KS):
                    nc.tensor.matmul(
                        ps_out[l][:],
                        exp_g[:, sc, ts(l, P)],
                        XB[:, b, sc, :],
                        start=(sc == 0),
                        stop=(sc == KS - 1),
                    )

            mm2(0)
            # transpose sums row -> per-label columns via K=1 matmuls
            ps_tiny = psum.tile([P, NSUB], F32, tag="ps_tiny", bufs=1)
            for l in range(NSUB):
                nc.tensor.matmul(
                    ps_tiny[:, l : l + 1], sums_row[:, ts(l, P)], one_11[:],
                    start=True, stop=True,
                )
            sums_col = work.tile([P, NSUB], F32, tag="sums_col")
            nc.vector.tensor_copy(sums_col[:], ps_tiny[:])
            recips = work.tile([P, NSUB], F32, tag="recips")
            nc.vector.reciprocal(recips[:], sums_col[:])

            def finish(l):
                out_t[l] = work.tile([P, h_sz], F32, tag="out", name="out_b")
                nc.scalar.activation(
                    out_t[l][:], ps_out[l][:],
                    mybir.ActivationFunctionType.Copy,
                    scale=recips[:, l : l + 1],
                )
                nc.sync.dma_start(
                    o_d[b, ds(g * LG + l * P, P), :], out_t[l][:]
                )

            finish(0)
            for l in range(1, NSUB):
                mm2(l)
                finish(l)

        mask_setup()
        x_setup(0)
        for g in range(min(2, NG)):
            w_setup(g)
        for g in range(NG):
            if g + 2 < NG:
                w_setup(g + 2)
            main_group(0, g)
        for b in range(1, b_sz):
            x_setup(b)
        for b in range(1, b_sz):
            for g in range(NG):
                main_group(b, g)

    nc.compile()
    return nc


_CACHE = {}

VARIANT = "a"  # "a": per-tile PE transposes (v7); "b": transposed-scores


def _get_module():
    if VARIANT not in _CACHE:
        _CACHE[VARIANT] = (
            build_module() if VARIANT == "a" else build_module_b()
        )
    return _CACHE[VARIANT]


def _run(inputs: np.ndarray, masks: np.ndarray, W: np.ndarray, **spmd_kwargs):
    """Run on 8 cores; returns (full output, BassKernelResults)."""
    nc = _get_module()

    x = np.ascontiguousarray(inputs, dtype=np.float32)
    xt = np.ascontiguousarray(np.swapaxes(x, 1, 2))
    mf = np.ascontiguousarray(masks, dtype=np.float32)
    wt_pad = np.zeros((H, L_PAD), dtype=np.float32)
    wt_pad[:, :L] = W.T

    in_maps = [
        {
            "x": x,
            "xt": xt,
            "m": mf,
            "wt": np.ascontiguousarray(wt_pad[:, c * LSH : (c + 1) * LSH]),
        }
        for c in range(N_CORES)
    ]
    res = bass_utils.run_bass_kernel_spmd(
        nc, in_maps, core_ids=list(range(N_CORES)), **spmd_kwargs
    )
    out = np.concatenate([res.results[c]["o"] for c in range(N_CORES)], axis=1)
    return np.ascontiguousarray(out[:, :L, :]), res


def kernel(inputs: np.ndarray, masks: np.ndarray, W: np.ndarray) -> np.ndarray:
    out, _ = _run(inputs, masks, W)
    return out
